# revision 9
# baseline (speedup 1.0000x reference)
"""GCN (3-layer, skip-concat) on 8 Trainium2 NeuronCores.

Strategy (hardcoded for N=10000, E=320000, dims 512/512/256):
  - Row-partition nodes across 8 cores (1280 padded rows each, N padded
    to 10240).
  - The adjacency shard is densified on the host into A_k^T [10240, 1280]
    (bf16): SpMM becomes a dense matmul on TensorE.
  - Activations live feature-major (X^T) in SBUF. Per layer:
      S_k   = X^T.T @ W          (node-major [1280, d_out], PE)
      S     = AllGather(S_k)     (bf16, HBM collective, 5x2MB chunks --
                                  measured CC per-byte cost is best ~2MB)
      Y^T   = S_tiles^T @ A_k^T  (feature-major, PE; S tiles stationary)
      X' ^T = [relu(Y^T + b); (Y^T + b)]   (partition-axis concat, free)
  - Software pipelining via emission order: layer L+1's dense m-tiles and
    all-gather chunks are emitted between layer L's SpMM n-chunks.
  - Two HWDGE queues: adjacency streams on qSP (sync); x tiles, weights,
    bounce writes, gathered-S loads and output on qAct (scalar), so the
    collective feeds never queue behind adjacency prefetch.
  - 6 physical s_ch buffers rotate over 3x5 logical chunks so the first
    gathered-S load of each layer has no write-after-read dependency on
    the previous layer's SpMM (kills the transition head-blocking).
  - DMAs are batched (1MB adjacency blocks, 4-mtile x blocks, single-DMA
    weights/biases) because each dma_start costs ~0.6us of issue time on
    the issuing engine.
  - xt1/xt2 share one SBUF buffer (their lifetimes are disjoint).
  - All matmuls bf16 with fp32 PSUM accumulation (rel err ~2e-3).
"""

import os
import numpy as np
from ml_dtypes import bfloat16

N = 10000
NPAD = 10240
NCORES = 8
R = NPAD // NCORES  # 1280 rows per core
P = 128
CT = NPAD // P  # 80 contraction tiles for the SpMM
KSUB = 4  # contraction subtiles per adjacency group
NGRP = CT // KSUB  # 20 groups
NBLK = NGRP // 2  # 10 two-group adjacency DMA blocks

NAG = 5  # all-gather chunks per layer
AGR = R // NAG  # 256 rows per rank per chunk
AGT = NPAD // NAG // P  # 16 gathered contraction tiles per chunk
NPHYS = 6  # physical s_ch buffers (one spare frees WAR at transitions)

D0_IN, D0_OUT = 512, 512
D1_IN, D1_OUT = 1024, 512
D2_IN, D2_OUT = 1024, 256

# n-chunks of the 1280-wide free dim (PSUM bank = 512 fp32)
N_CHUNKS = [(0, 512), (512, 512), (1024, 256)]
# dense m-tiles whose lhsT columns come from n-chunk i's epilogue
M_OF_NC = [(0, 1, 2, 3), (4, 5, 6, 7), (8, 9)]
# all-gather chunk j consumes dense m-tiles 2j, 2j+1
AG_OF_NC = [(0, 1), (2, 3), (4,)]

XBLK = 4  # m-tiles per x DMA block
NXBLK = CT // XBLK  # 20

_CACHE = {}
LAST_RESULT = None  # BassKernelResults of the most recent run (for test.py)


def _phys(L, j):
    return (NAG * L + j) % NPHYS


def _build_bass():
    import concourse.bass as bass
    import concourse.bacc as bacc
    import concourse.mybir as mybir
    import concourse.tile as tile

    dt = mybir.dt
    bf16 = dt.bfloat16
    f32 = dt.float32
    ts = bass.ts

    nc = bacc.Bacc(
        "TRN2",
        target_bir_lowering=False,
        debug=False,
        enable_asserts=False,
        num_devices=NCORES,
    )

    xTf_d = nc.dram_tensor(
        "xTf", [NXBLK, P, XBLK, D0_IN // P, P], bf16, kind="ExternalInput"
    )
    # pre-tiled adjacency, 1MB blocks of 2 groups x [P, KSUB, nw]
    adjT01_d = nc.dram_tensor(
        "adjT01", [2, NBLK, P, 2, KSUB, 512], bf16, kind="ExternalInput"
    )
    adjT2_d = nc.dram_tensor(
        "adjT2", [NBLK, P, 2, KSUB, 256], bf16, kind="ExternalInput"
    )
    W_d = [
        nc.dram_tensor("W0", [P, D0_IN // P, D0_OUT], bf16, kind="ExternalInput"),
        nc.dram_tensor("W1", [P, D1_IN // P, D1_OUT], bf16, kind="ExternalInput"),
        nc.dram_tensor("W2", [P, D2_IN // P, D2_OUT], bf16, kind="ExternalInput"),
    ]
    b_d = [
        nc.dram_tensor("b0", [P, D0_OUT // P], f32, kind="ExternalInput"),
        nc.dram_tensor("b1", [P, D1_OUT // P], f32, kind="ExternalInput"),
        nc.dram_tensor("b2", [P, D2_OUT // P], f32, kind="ExternalInput"),
    ]
    # [p, t, r]: feature f = t*P + p of output row r (decoded host-side)
    outT_d = nc.dram_tensor("outT", [P, D2_OUT // P, R], f32, kind="ExternalOutput")

    DIMS = [(D0_IN, D0_OUT), (D1_IN, D1_OUT), (D2_IN, D2_OUT)]

    with tile.TileContext(nc) as tc:
        ctx_pools = (
            tc.tile_pool(name="persist", bufs=1),
            tc.tile_pool(name="work", bufs=3),
            tc.tile_pool(name="psum", bufs=1, space="PSUM"),
            tc.tile_pool(name="dram", bufs=1, space="DRAM"),
        )
        with ctx_pools[0] as persist, ctx_pools[1] as work, \
             ctx_pools[2] as psum_pool, ctx_pools[3] as dram_pool:

            # ---- persistent tiles ----
            w_sb = [
                persist.tile([P, d_in // P, d_out], bf16, name=f"w{L}", tag=f"w{L}")
                for L, (d_in, d_out) in enumerate(DIMS)
            ]
            b_sb = [
                persist.tile([P, d_out // P], f32, name=f"b{L}", tag=f"b{L}")
                for L, (_, d_out) in enumerate(DIMS)
            ]
            # activations X^T (feature-major); xt1/xt2 lifetimes are
            # disjoint so they share one buffer
            xt = persist.tile([P, D1_IN // P, R], bf16, name="xt", tag="xt")
            xts = [None, xt, xt]

            # gathered S: 6 physical buffers rotating over 5 chunks/layer
            s_ch = [
                persist.tile([P, AGT, 512], bf16, name=f"s_ch{b}", tag=f"s_ch{b}")
                for b in range(NPHYS)
            ]

            s_bounce = [
                dram_pool.tile([R, d_out], bf16, name=f"s_bounce{L}", tag=f"sb{L}")
                for L, (_, d_out) in enumerate(DIMS)
            ]
            s_all = [
                [
                    dram_pool.tile(
                        [NCORES * AGR, d_out],
                        bf16,
                        name=f"s_all{L}_{j}",
                        tag=f"sa{L}_{j}",
                        addr_space="Shared",
                    )
                    for j in range(NAG)
                ]
                for L, (_, d_out) in enumerate(DIMS)
            ]

            # ---- startup: first x blocks interleaved with w0, rest after ----
            xtiles = {}

            def load_xblk(blk):
                t = work.tile(
                    [P, XBLK, D0_IN // P, P], bf16, name=f"xtile_{blk}",
                    tag="xtile", bufs=3,
                )
                nc.scalar.dma_start(t[:], xTf_d[blk])
                xtiles[blk] = t

            load_xblk(0)
            nc.sync.dma_start(w_sb[0][:], W_d[0][:])
            load_xblk(1)
            nc.sync.dma_start(w_sb[1][:], W_d[1][:])
            nc.sync.dma_start(w_sb[2][:], W_d[2][:])
            for L in range(3):
                nc.sync.dma_start(b_sb[L][:], b_d[L][:])

            def dense_m(L, m):
                """dense S_k m-tile: psum = xt.T @ W, cast to bf16, to bounce."""
                d_in, d_out = DIMS[L]
                n_ct = d_in // P
                dps = psum_pool.tile(
                    [P, d_out], f32, name=f"dps_{L}_{m}", tag="dense_ps", bufs=2
                )
                for c in range(n_ct):
                    nc.tensor.matmul(
                        dps[:],
                        lhsT=xts[L][:, c, ts(m, P)],
                        rhs=w_sb[L][:, c, :],
                        start=(c == 0),
                        stop=(c == n_ct - 1),
                    )
                s_sb = work.tile(
                    [P, d_out], bf16, name=f"ssb_{L}_{m}", tag="s_sb", bufs=4
                )
                nc.vector.tensor_copy(s_sb[:], dps[:])
                # bounce writes ride the PE-paced sync queue: the scalar queue
                # head-blocks on collective waits (s_loads) and would convoy
                # these, delaying the next all-gather issue
                nc.sync.dma_start(s_bounce[L][ts(m, P), :], s_sb[:])

            def ag_issue(L, j):
                """all-gather chunk j of layer L's S (writes s_all only)."""
                nc.gpsimd.collective_compute(
                    "AllGather",
                    mybir.AluOpType.bypass,
                    replica_groups=[list(range(NCORES))],
                    ins=[s_bounce[L][ts(j, AGR), :].opt()],
                    outs=[s_all[L][j].opt()],
                )

            def s_load(L, j):
                """load gathered chunk j into its physical buffer. Emitted
                after the program-order readers of that buffer (layer L-1)."""
                d_out = DIMS[L][1]
                src = s_all[L][j].rearrange("(t p) d -> p t d", p=P)
                nc.scalar.dma_start(s_ch[_phys(L, j)][:, :, :d_out], src)

            def spmm_nc(L, nci, sink):
                """SpMM n-chunk nci of layer L + epilogue via sink()."""
                d_out = DIMS[L][1]
                n_po = d_out // P
                n0, nw = N_CHUNKS[nci]
                sp_ps = [
                    psum_pool.tile(
                        [P, nw], f32, name=f"sp_{L}_{nci}_{p}", tag=f"sp{p}"
                    )
                    for p in range(n_po)
                ]
                for b in range(NBLK):
                    if nci < 2:
                        at = work.tile(
                            [P, 2, KSUB, 512], bf16,
                            name=f"at_{L}_{nci}_{b}", tag="at", bufs=3,
                        )
                        nc.sync.dma_start(at[:], adjT01_d[nci, b])
                    else:
                        at = work.tile(
                            [P, 2, KSUB, 256], bf16,
                            name=f"at2_{L}_{b}", tag="at2", bufs=3,
                        )
                        nc.sync.dma_start(at[:], adjT2_d[b])
                    for g2 in range(2):
                        gg = 2 * b + g2
                        jc = gg // 4  # all-gather chunk of this group
                        tbase = (gg % 4) * KSUB
                        sch = s_ch[_phys(L, jc)]
                        for s in range(KSUB):
                            for p in range(n_po):
                                nc.tensor.matmul(
                                    sp_ps[p][:],
                                    lhsT=sch[:, tbase + s, ts(p, P)],
                                    rhs=at[:, g2, s, :],
                                    start=(b == 0 and g2 == 0 and s == 0),
                                    stop=(
                                        b == NBLK - 1 and g2 == 1 and s == KSUB - 1
                                    ),
                                )
                for p in range(n_po):
                    sink(p, sp_ps[p], n0, nw, n_po)

            def sink_mid(L):
                n_po_out = DIMS[L][1] // P

                def sink(p, ps, n0, nw, n_po):
                    # relu(x+b) on Vector (not ScalarE): the scalar queue
                    # head-blocks on s_load collective waits and would convoy
                    # an ACTIVATE sitting behind them
                    nc.vector.tensor_scalar(
                        xts[L + 1][:, p, n0 : n0 + nw],
                        ps[:],
                        b_sb[L][:, p : p + 1],
                        0.0,
                        op0=mybir.AluOpType.add,
                        op1=mybir.AluOpType.max,
                    )
                    nc.vector.tensor_scalar_add(
                        xts[L + 1][:, n_po_out + p, n0 : n0 + nw],
                        ps[:],
                        b_sb[L][:, p : p + 1],
                    )

                return sink

            def make_sink_out():
                state = {}

                def sink(p, ps, n0, nw, n_po):
                    if p == 0:
                        state["ot"] = work.tile(
                            [P, n_po, nw], f32, name=f"ot_{n0}", tag="ot", bufs=2
                        )
                    ot = state["ot"]
                    nc.vector.tensor_scalar_add(
                        ot[:, p, :], ps[:], b_sb[2][:, p : p + 1]
                    )
                    if p == n_po - 1:
                        nc.scalar.dma_start(outT_d[:, :, n0 : n0 + nw], ot[:])

                return sink

            # ================= pipeline =================
            # layer 0: every core computes the FULL S0 = x @ W0 locally
            # (redundant across cores) straight into s_ch -- no collective,
            # so the first all-gather is layer 1's, which has pipeline slack.
            for blk in range(NXBLK):
                if blk + 2 < NXBLK:
                    load_xblk(blk + 2)
                xtile = xtiles.pop(blk)
                for mi in range(XBLK):
                    mt = XBLK * blk + mi
                    dps = psum_pool.tile(
                        [P, D0_OUT], f32, name=f"dps0_{mt}", tag="dense_ps", bufs=2
                    )
                    for c in range(D0_IN // P):
                        nc.tensor.matmul(
                            dps[:],
                            lhsT=xtile[:, mi, c, :],
                            rhs=w_sb[0][:, c, :],
                            start=(c == 0),
                            stop=(c == D0_IN // P - 1),
                        )
                    nc.vector.tensor_copy(
                        s_ch[_phys(0, mt // AGT)][:, mt % AGT, :], dps[:]
                    )
            # layer L spmm interleaved with layer L+1 dense + gather issue;
            # the s_ch loads must come after L's last spmm reads (program
            # order = Tile trace order), so they sit before L+1's spmm.
            for L in (0, 1):
                for nci in range(3):
                    spmm_nc(L, nci, sink_mid(L))
                    for m in M_OF_NC[nci]:
                        dense_m(L + 1, m)
                    for j in AG_OF_NC[nci]:
                        ag_issue(L + 1, j)
                for j in range(NAG):
                    s_load(L + 1, j)
            for nci in range(3):
                spmm_nc(2, nci, make_sink_out())

    nc.compile()
    return nc


def _get_nc():
    if "nc" not in _CACHE:
        _CACHE["nc"] = _build_bass()
    return _CACHE["nc"]


def _preprocess(x, edge_row, edge_col, edge_val, W0, W1, W2, b0, b1, b2):
    x = np.asarray(x, np.float32)
    edge_row = np.asarray(edge_row, np.int64)
    edge_col = np.asarray(edge_col, np.int64)
    edge_val = np.asarray(edge_val, np.float32)

    # contraction permutation for the chunked all-gather:
    # new index j*2048 + k*256 + r  <->  old (global node) k*1280 + j*256 + r
    jj, kk, rr = np.meshgrid(
        np.arange(NAG), np.arange(NCORES), np.arange(AGR), indexing="ij"
    )
    new_of_old = np.empty(NPAD, np.int64)
    new_of_old[(kk * R + jj * AGR + rr).ravel()] = (
        jj * (NCORES * AGR) + kk * AGR + rr
    ).ravel()

    # dense per-core adjacency blocks, transposed + permuted:
    # adjT[k][new_of_old[c], r_local] = sum of vals of edges (k*R+r_local, c)
    adjT = np.zeros((NCORES, NPAD, R), np.float32)
    core = edge_row // R
    r_local = edge_row % R
    np.add.at(adjT, (core, new_of_old[edge_col], r_local), edge_val)
    adjT = adjT.astype(bfloat16)
    # [cores, NBLK, 2, KSUB, P, R] -> [cores, NBLK, P, 2, KSUB, R]
    a6 = adjT.reshape(NCORES, NBLK, 2, KSUB, P, R).transpose(0, 1, 4, 2, 3, 5)
    adjT01 = np.ascontiguousarray(
        np.stack([a6[..., 0:512], a6[..., 512:1024]], axis=1)
    )  # [cores, 2, NBLK, P, 2, KSUB, 512]
    adjT2 = np.ascontiguousarray(a6[..., 1024:1280])  # [cores, NBLK, P, 2, KSUB, 256]

    x_pad = np.zeros((NPAD, x.shape[1]), np.float32)
    x_pad[:N] = x
    old_of_new = np.empty(NPAD, np.int64)
    old_of_new[new_of_old] = np.arange(NPAD)
    # [blk, mi, n(row-in-tile), c, pf] -> [blk, pf, mi, c, n]
    xp5 = x_pad[old_of_new].reshape(NXBLK, XBLK, P, D0_IN // P, P)
    xTf = np.ascontiguousarray(xp5.transpose(0, 4, 1, 3, 2)).astype(bfloat16)

    def wtile(W):
        W = np.asarray(W, np.float32)
        d_in, d_out = W.shape
        return np.ascontiguousarray(
            W.reshape(d_in // P, P, d_out).transpose(1, 0, 2)
        ).astype(bfloat16)

    def btile(b):
        b = np.asarray(b, np.float32)
        return np.ascontiguousarray(b.reshape(-1, P).T)

    in_maps = []
    for k in range(NCORES):
        in_maps.append(
            {
                "xTf": xTf,
                "adjT01": adjT01[k],
                "adjT2": adjT2[k],
                "W0": wtile(W0),
                "W1": wtile(W1),
                "W2": wtile(W2),
                "b0": btile(b0),
                "b1": btile(b1),
                "b2": btile(b2),
            }
        )
    return in_maps


def kernel(x, edge_row, edge_col, edge_val, W0, W1, W2, b0, b1, b2):
    global LAST_RESULT
    from concourse.bass_utils import run_bass_kernel_spmd

    nc = _get_nc()
    in_maps = _preprocess(
        x, edge_row, edge_col, edge_val, W0, W1, W2, b0, b1, b2
    )
    res = run_bass_kernel_spmd(
        nc,
        in_maps,
        core_ids=list(range(NCORES)),
        trace=bool(int(os.environ.get("GCN_TRACE", "0"))),
    )
    LAST_RESULT = res

    # per-core outT is [P, 2, R]; feature f = t*P + p
    outT = np.concatenate(
        [
            np.asarray(res.results[k]["outT"]).transpose(1, 0, 2).reshape(D2_OUT, R)
            for k in range(NCORES)
        ],
        axis=1,
    )  # [256, 10240]
    return np.ascontiguousarray(outT.T[:N]).astype(np.float32)


# revision 10
# speedup vs baseline: 1.0210x; 1.0210x over previous
"""GCN (3-layer, skip-concat) on 8 Trainium2 NeuronCores.

Strategy (hardcoded for N=10000, E=320000, dims 512/512/256):
  - Row-partition nodes across 8 cores (1280 padded rows each, N padded
    to 10240).
  - The adjacency shard is densified on the host into A_k^T [10240, 1280]
    (bf16): SpMM becomes a dense matmul on TensorE.
  - Activations live feature-major (X^T) in SBUF. Per layer:
      S_k   = X^T.T @ W          (node-major [1280, d_out], PE)
      S     = AllGather(S_k)     (bf16, HBM collective, 5x2MB chunks --
                                  measured CC per-byte cost is best ~2MB)
      Y^T   = S_tiles^T @ A_k^T  (feature-major, PE; S tiles stationary)
      X' ^T = [relu(Y^T + b); (Y^T + b)]   (partition-axis concat, free)
  - Software pipelining via emission order: layer L+1's dense m-tiles and
    all-gather chunks are emitted between layer L's SpMM n-chunks.
  - Two HWDGE queues: adjacency streams on qSP (sync); x tiles, weights,
    bounce writes, gathered-S loads and output on qAct (scalar), so the
    collective feeds never queue behind adjacency prefetch.
  - 6 physical s_ch buffers rotate over 3x5 logical chunks so the first
    gathered-S load of each layer has no write-after-read dependency on
    the previous layer's SpMM (kills the transition head-blocking).
  - DMAs are batched (1MB adjacency blocks, 4-mtile x blocks, single-DMA
    weights/biases) because each dma_start costs ~0.6us of issue time on
    the issuing engine.
  - xt1/xt2 share one SBUF buffer (their lifetimes are disjoint).
  - All matmuls bf16 with fp32 PSUM accumulation (rel err ~2e-3).
"""

import os
import numpy as np
from ml_dtypes import bfloat16

N = 10000
NPAD = 10240
NCORES = 8
R = NPAD // NCORES  # 1280 rows per core
P = 128
CT = NPAD // P  # 80 contraction tiles for the SpMM
KSUB = 4  # contraction subtiles per adjacency group
NGRP = CT // KSUB  # 20 groups
NBLK = NGRP // 2  # 10 two-group adjacency DMA blocks

NAG = 5  # all-gather chunks per layer
AGR = R // NAG  # 256 rows per rank per chunk
AGT = NPAD // NAG // P  # 16 gathered contraction tiles per chunk
NPHYS = 6  # physical s_ch buffers (one spare frees WAR at transitions)

D0_IN, D0_OUT = 512, 512
D1_IN, D1_OUT = 1024, 512
D2_IN, D2_OUT = 1024, 256

# n-chunks of the 1280-wide free dim (PSUM bank = 512 fp32)
N_CHUNKS = [(0, 512), (512, 512), (1024, 256)]
# dense m-tiles whose lhsT columns come from n-chunk i's epilogue
M_OF_NC = [(0, 1, 2, 3), (4, 5, 6, 7), (8, 9)]
# all-gather chunk j consumes dense m-tiles 2j, 2j+1
AG_OF_NC = [(0, 1), (2, 3), (4,)]

XBLK = 4  # m-tiles per x DMA block
NXBLK = CT // XBLK  # 20

_CACHE = {}
LAST_RESULT = None  # BassKernelResults of the most recent run (for test.py)


def _phys(L, j):
    return (NAG * L + j) % NPHYS


def _build_bass():
    import concourse.bass as bass
    import concourse.bacc as bacc
    import concourse.mybir as mybir
    import concourse.tile as tile

    dt = mybir.dt
    bf16 = dt.bfloat16
    f32 = dt.float32
    ts = bass.ts

    nc = bacc.Bacc(
        "TRN2",
        target_bir_lowering=False,
        debug=False,
        enable_asserts=False,
        num_devices=NCORES,
    )

    xTf_d = nc.dram_tensor(
        "xTf", [NXBLK, P, XBLK, D0_IN // P, P], bf16, kind="ExternalInput"
    )
    # pre-tiled adjacency, 1MB blocks of 2 groups x [P, KSUB, nw]
    adjT01_d = nc.dram_tensor(
        "adjT01", [2, NBLK, P, 2, KSUB, 512], bf16, kind="ExternalInput"
    )
    adjT2_d = nc.dram_tensor(
        "adjT2", [NBLK, P, 2, KSUB, 256], bf16, kind="ExternalInput"
    )
    W_d = [
        nc.dram_tensor("W0", [P, D0_IN // P, D0_OUT], bf16, kind="ExternalInput"),
        nc.dram_tensor("W1", [P, D1_IN // P, D1_OUT], bf16, kind="ExternalInput"),
        nc.dram_tensor("W2", [P, D2_IN // P, D2_OUT], bf16, kind="ExternalInput"),
    ]
    b_d = [
        nc.dram_tensor("b0", [P, D0_OUT // P], f32, kind="ExternalInput"),
        nc.dram_tensor("b1", [P, D1_OUT // P], f32, kind="ExternalInput"),
        nc.dram_tensor("b2", [P, D2_OUT // P], f32, kind="ExternalInput"),
    ]
    # [p, t, r]: feature f = t*P + p of output row r (decoded host-side)
    outT_d = nc.dram_tensor("outT", [P, D2_OUT // P, R], f32, kind="ExternalOutput")

    DIMS = [(D0_IN, D0_OUT), (D1_IN, D1_OUT), (D2_IN, D2_OUT)]

    with tile.TileContext(nc) as tc:
        ctx_pools = (
            tc.tile_pool(name="persist", bufs=1),
            tc.tile_pool(name="work", bufs=3),
            tc.tile_pool(name="psum", bufs=1, space="PSUM"),
            tc.tile_pool(name="dram", bufs=1, space="DRAM"),
        )
        with ctx_pools[0] as persist, ctx_pools[1] as work, \
             ctx_pools[2] as psum_pool, ctx_pools[3] as dram_pool:

            # ---- persistent tiles ----
            w_sb = [
                persist.tile([P, d_in // P, d_out], bf16, name=f"w{L}", tag=f"w{L}")
                for L, (d_in, d_out) in enumerate(DIMS)
            ]
            b_sb = [
                persist.tile([P, d_out // P], f32, name=f"b{L}", tag=f"b{L}")
                for L, (_, d_out) in enumerate(DIMS)
            ]
            # activations X^T (feature-major); xt1/xt2 lifetimes are
            # disjoint so they share one buffer
            xt = persist.tile([P, D1_IN // P, R], bf16, name="xt", tag="xt")
            xts = [None, xt, xt]

            # gathered S: 6 physical buffers rotating over 5 chunks/layer
            s_ch = [
                persist.tile([P, AGT, 512], bf16, name=f"s_ch{b}", tag=f"s_ch{b}")
                for b in range(NPHYS)
            ]

            s_bounce = [
                dram_pool.tile([R, d_out], bf16, name=f"s_bounce{L}", tag=f"sb{L}")
                for L, (_, d_out) in enumerate(DIMS)
            ]
            s_all = [
                [
                    dram_pool.tile(
                        [NCORES * AGR, d_out],
                        bf16,
                        name=f"s_all{L}_{j}",
                        tag=f"sa{L}_{j}",
                        addr_space="Shared",
                    )
                    for j in range(NAG)
                ]
                for L, (_, d_out) in enumerate(DIMS)
            ]

            # ---- startup: first x blocks interleaved with w0, rest after ----
            xtiles = {}

            def load_xblk(blk):
                t = work.tile(
                    [P, XBLK, D0_IN // P, P], bf16, name=f"xtile_{blk}",
                    tag="xtile", bufs=3,
                )
                nc.scalar.dma_start(t[:], xTf_d[blk])
                xtiles[blk] = t

            load_xblk(0)
            nc.sync.dma_start(w_sb[0][:], W_d[0][:])
            load_xblk(1)
            nc.sync.dma_start(w_sb[1][:], W_d[1][:])
            nc.sync.dma_start(w_sb[2][:], W_d[2][:])
            for L in range(3):
                nc.sync.dma_start(b_sb[L][:], b_d[L][:])

            def dense_m(L, m):
                """dense S_k m-tile: psum = xt.T @ W, cast to bf16, to bounce."""
                d_in, d_out = DIMS[L]
                n_ct = d_in // P
                dps = psum_pool.tile(
                    [P, d_out], f32, name=f"dps_{L}_{m}", tag="dense_ps", bufs=2
                )
                for c in range(n_ct):
                    nc.tensor.matmul(
                        dps[:],
                        lhsT=xts[L][:, c, ts(m, P)],
                        rhs=w_sb[L][:, c, :],
                        start=(c == 0),
                        stop=(c == n_ct - 1),
                    )
                s_sb = work.tile(
                    [P, d_out], bf16, name=f"ssb_{L}_{m}", tag="s_sb", bufs=4
                )
                nc.vector.tensor_copy(s_sb[:], dps[:])
                nc.scalar.dma_start(s_bounce[L][ts(m, P), :], s_sb[:])

            def ag_issue(L, j):
                """all-gather chunk j of layer L's S (writes s_all only)."""
                nc.gpsimd.collective_compute(
                    "AllGather",
                    mybir.AluOpType.bypass,
                    replica_groups=[list(range(NCORES))],
                    ins=[s_bounce[L][ts(j, AGR), :].opt()],
                    outs=[s_all[L][j].opt()],
                )

            def s_load(L, j):
                """load gathered chunk j into its physical buffer. Emitted
                after the program-order readers of that buffer (layer L-1)."""
                d_out = DIMS[L][1]
                src = s_all[L][j].rearrange("(t p) d -> p t d", p=P)
                nc.scalar.dma_start(s_ch[_phys(L, j)][:, :, :d_out], src)

            def spmm_nc(L, nci, sink):
                """SpMM n-chunk nci of layer L + epilogue via sink()."""
                d_out = DIMS[L][1]
                n_po = d_out // P
                n0, nw = N_CHUNKS[nci]
                sp_ps = [
                    psum_pool.tile(
                        [P, nw], f32, name=f"sp_{L}_{nci}_{p}", tag=f"sp{p}"
                    )
                    for p in range(n_po)
                ]
                for b in range(NBLK):
                    if nci < 2:
                        at = work.tile(
                            [P, 2, KSUB, 512], bf16,
                            name=f"at_{L}_{nci}_{b}", tag="at", bufs=3,
                        )
                        nc.sync.dma_start(at[:], adjT01_d[nci, b])
                    else:
                        at = work.tile(
                            [P, 2, KSUB, 256], bf16,
                            name=f"at2_{L}_{b}", tag="at2", bufs=3,
                        )
                        nc.sync.dma_start(at[:], adjT2_d[b])
                    for g2 in range(2):
                        gg = 2 * b + g2
                        jc = gg // 4  # all-gather chunk of this group
                        tbase = (gg % 4) * KSUB
                        sch = s_ch[_phys(L, jc)]
                        for s in range(KSUB):
                            for p in range(n_po):
                                nc.tensor.matmul(
                                    sp_ps[p][:],
                                    lhsT=sch[:, tbase + s, ts(p, P)],
                                    rhs=at[:, g2, s, :],
                                    start=(b == 0 and g2 == 0 and s == 0),
                                    stop=(
                                        b == NBLK - 1 and g2 == 1 and s == KSUB - 1
                                    ),
                                )
                for p in range(n_po):
                    sink(p, sp_ps[p], n0, nw, n_po)

            def sink_mid(L):
                n_po_out = DIMS[L][1] // P

                def sink(p, ps, n0, nw, n_po):
                    # relu(x+b) on Vector (not ScalarE): the scalar queue
                    # head-blocks on s_load collective waits and would convoy
                    # an ACTIVATE sitting behind them
                    nc.vector.tensor_scalar(
                        xts[L + 1][:, p, n0 : n0 + nw],
                        ps[:],
                        b_sb[L][:, p : p + 1],
                        0.0,
                        op0=mybir.AluOpType.add,
                        op1=mybir.AluOpType.max,
                    )
                    nc.vector.tensor_scalar_add(
                        xts[L + 1][:, n_po_out + p, n0 : n0 + nw],
                        ps[:],
                        b_sb[L][:, p : p + 1],
                    )

                return sink

            def make_sink_out():
                state = {}

                def sink(p, ps, n0, nw, n_po):
                    if p == 0:
                        state["ot"] = work.tile(
                            [P, n_po, nw], f32, name=f"ot_{n0}", tag="ot", bufs=2
                        )
                    ot = state["ot"]
                    nc.vector.tensor_scalar_add(
                        ot[:, p, :], ps[:], b_sb[2][:, p : p + 1]
                    )
                    if p == n_po - 1:
                        nc.scalar.dma_start(outT_d[:, :, n0 : n0 + nw], ot[:])

                return sink

            # ================= pipeline =================
            # layer 0: every core computes the FULL S0 = x @ W0 locally
            # (redundant across cores) straight into s_ch -- no collective,
            # so the first all-gather is layer 1's, which has pipeline slack.
            for blk in range(NXBLK):
                if blk + 2 < NXBLK:
                    load_xblk(blk + 2)
                xtile = xtiles.pop(blk)
                for mi in range(XBLK):
                    mt = XBLK * blk + mi
                    dps = psum_pool.tile(
                        [P, D0_OUT], f32, name=f"dps0_{mt}", tag="dense_ps", bufs=2
                    )
                    for c in range(D0_IN // P):
                        nc.tensor.matmul(
                            dps[:],
                            lhsT=xtile[:, mi, c, :],
                            rhs=w_sb[0][:, c, :],
                            start=(c == 0),
                            stop=(c == D0_IN // P - 1),
                        )
                    nc.vector.tensor_copy(
                        s_ch[_phys(0, mt // AGT)][:, mt % AGT, :], dps[:]
                    )
            # layer L spmm interleaved with layer L+1 dense + gather issue;
            # the s_ch loads must come after L's last spmm reads (program
            # order = Tile trace order), so they sit before L+1's spmm.
            for L in (0, 1):
                for nci in range(3):
                    spmm_nc(L, nci, sink_mid(L))
                    for m in M_OF_NC[nci]:
                        dense_m(L + 1, m)
                    for j in AG_OF_NC[nci]:
                        ag_issue(L + 1, j)
                for j in range(NAG):
                    s_load(L + 1, j)
            for nci in range(3):
                spmm_nc(2, nci, make_sink_out())

    nc.compile()
    return nc


def _get_nc():
    if "nc" not in _CACHE:
        _CACHE["nc"] = _build_bass()
    return _CACHE["nc"]


def _preprocess(x, edge_row, edge_col, edge_val, W0, W1, W2, b0, b1, b2):
    x = np.asarray(x, np.float32)
    edge_row = np.asarray(edge_row, np.int64)
    edge_col = np.asarray(edge_col, np.int64)
    edge_val = np.asarray(edge_val, np.float32)

    # contraction permutation for the chunked all-gather:
    # new index j*2048 + k*256 + r  <->  old (global node) k*1280 + j*256 + r
    jj, kk, rr = np.meshgrid(
        np.arange(NAG), np.arange(NCORES), np.arange(AGR), indexing="ij"
    )
    new_of_old = np.empty(NPAD, np.int64)
    new_of_old[(kk * R + jj * AGR + rr).ravel()] = (
        jj * (NCORES * AGR) + kk * AGR + rr
    ).ravel()

    # dense per-core adjacency blocks, transposed + permuted:
    # adjT[k][new_of_old[c], r_local] = sum of vals of edges (k*R+r_local, c)
    adjT = np.zeros((NCORES, NPAD, R), np.float32)
    core = edge_row // R
    r_local = edge_row % R
    np.add.at(adjT, (core, new_of_old[edge_col], r_local), edge_val)
    adjT = adjT.astype(bfloat16)
    # [cores, NBLK, 2, KSUB, P, R] -> [cores, NBLK, P, 2, KSUB, R]
    a6 = adjT.reshape(NCORES, NBLK, 2, KSUB, P, R).transpose(0, 1, 4, 2, 3, 5)
    adjT01 = np.ascontiguousarray(
        np.stack([a6[..., 0:512], a6[..., 512:1024]], axis=1)
    )  # [cores, 2, NBLK, P, 2, KSUB, 512]
    adjT2 = np.ascontiguousarray(a6[..., 1024:1280])  # [cores, NBLK, P, 2, KSUB, 256]

    x_pad = np.zeros((NPAD, x.shape[1]), np.float32)
    x_pad[:N] = x
    old_of_new = np.empty(NPAD, np.int64)
    old_of_new[new_of_old] = np.arange(NPAD)
    # [blk, mi, n(row-in-tile), c, pf] -> [blk, pf, mi, c, n]
    xp5 = x_pad[old_of_new].reshape(NXBLK, XBLK, P, D0_IN // P, P)
    xTf = np.ascontiguousarray(xp5.transpose(0, 4, 1, 3, 2)).astype(bfloat16)

    def wtile(W):
        W = np.asarray(W, np.float32)
        d_in, d_out = W.shape
        return np.ascontiguousarray(
            W.reshape(d_in // P, P, d_out).transpose(1, 0, 2)
        ).astype(bfloat16)

    def btile(b):
        b = np.asarray(b, np.float32)
        return np.ascontiguousarray(b.reshape(-1, P).T)

    in_maps = []
    for k in range(NCORES):
        in_maps.append(
            {
                "xTf": xTf,
                "adjT01": adjT01[k],
                "adjT2": adjT2[k],
                "W0": wtile(W0),
                "W1": wtile(W1),
                "W2": wtile(W2),
                "b0": btile(b0),
                "b1": btile(b1),
                "b2": btile(b2),
            }
        )
    return in_maps


def kernel(x, edge_row, edge_col, edge_val, W0, W1, W2, b0, b1, b2):
    global LAST_RESULT
    from concourse.bass_utils import run_bass_kernel_spmd

    nc = _get_nc()
    in_maps = _preprocess(
        x, edge_row, edge_col, edge_val, W0, W1, W2, b0, b1, b2
    )
    res = run_bass_kernel_spmd(
        nc,
        in_maps,
        core_ids=list(range(NCORES)),
        trace=bool(int(os.environ.get("GCN_TRACE", "0"))),
    )
    LAST_RESULT = res

    # per-core outT is [P, 2, R]; feature f = t*P + p
    outT = np.concatenate(
        [
            np.asarray(res.results[k]["outT"]).transpose(1, 0, 2).reshape(D2_OUT, R)
            for k in range(NCORES)
        ],
        axis=1,
    )  # [256, 10240]
    return np.ascontiguousarray(outT.T[:N]).astype(np.float32)


# revision 21
# speedup vs baseline: 1.1022x; 1.0795x over previous
"""GCN (3-layer, skip-concat) on 8 Trainium2 NeuronCores.

Strategy (hardcoded for N=10000, E=320000, dims 512/512/256):
  - Row-partition nodes across 8 cores (1280 padded rows each, N padded
    to 10240).
  - The adjacency shard is densified on the host into A_k^T [10240, 1280]
    (bf16): SpMM becomes a dense matmul on TensorE.
  - Activations live feature-major (X^T) in SBUF. Per layer:
      S_k   = X^T.T @ W          (node-major [1280, d_out], PE)
      S     = AllGather(S_k)     (bf16, HBM collective, 5x2MB chunks --
                                  measured CC per-byte cost is best ~2MB)
      Y^T   = S_tiles^T @ A_k^T  (feature-major, PE; S tiles stationary)
      X' ^T = [relu(Y^T + b); (Y^T + b)]   (partition-axis concat, free)
  - Software pipelining via emission order: layer L+1's dense m-tiles and
    all-gather chunks are emitted between layer L's SpMM n-chunks.
  - Two HWDGE queues: adjacency streams on qSP (sync); x tiles, weights,
    bounce writes, gathered-S loads and output on qAct (scalar), so the
    collective feeds never queue behind adjacency prefetch.
  - 6 physical s_ch buffers rotate over 3x5 logical chunks so the first
    gathered-S load of each layer has no write-after-read dependency on
    the previous layer's SpMM (kills the transition head-blocking).
  - DMAs are batched (1MB adjacency blocks, 4-mtile x blocks, single-DMA
    weights/biases) because each dma_start costs ~0.6us of issue time on
    the issuing engine.
  - xt1/xt2 share one SBUF buffer (their lifetimes are disjoint).
  - SpMM runs in fp8 DoubleRow mode at 2x PE rate: the adjacency is e4m3
    (x4 pre-scale, 1/4 folded into W), and S rides dual-rail as
    (hi, lo) e4m3 pairs with hi+lo ~ bf16 precision, so only the
    adjacency quantization (~1% rms) hits accuracy. The rhs pair dim is
    a 0-stride broadcast of the same adjacency block, so fp8 also halves
    adjacency DMA traffic. Dense X@W matmuls stay bf16.
    (measured end-to-end rel err ~1.1e-2 vs the 2e-2 gate)
"""

import os
import numpy as np
from ml_dtypes import bfloat16, float8_e4m3fn

N = 10000
NPAD = 10240
NCORES = 8
R = NPAD // NCORES  # 1280 rows per core
P = 128
CT = NPAD // P  # 80 contraction tiles for the SpMM
KSUB = 4  # contraction subtiles per adjacency group
NGRP = CT // KSUB  # 20 groups
NBLK = NGRP // 2  # 10 two-group adjacency DMA blocks

NAG = 5  # all-gather chunks per layer
AGR = R // NAG  # 256 rows per rank per chunk
AGT = NPAD // NAG // P  # 16 gathered contraction tiles per chunk
NPHYS = 6  # physical s_ch buffers (one spare frees WAR at transitions)

D0_IN, D0_OUT = 512, 512
D1_IN, D1_OUT = 1024, 512
D2_IN, D2_OUT = 1024, 256

# n-chunks of the 1280-wide free dim (PSUM bank = 512 fp32)
N_CHUNKS = [(0, 512), (512, 512), (1024, 256)]
# dense m-tiles whose lhsT columns come from n-chunk i's epilogue
M_OF_NC = [(0, 1, 2, 3), (4, 5, 6, 7), (8, 9)]
# all-gather chunk j consumes dense m-tiles 2j, 2j+1
AG_OF_NC = [(0, 1), (2, 3), (4,)]

XBLK = 4  # m-tiles per x DMA block
NXBLK = CT // XBLK  # 20

_CACHE = {}
LAST_RESULT = None  # BassKernelResults of the most recent run (for test.py)


def _phys(L, j):
    return (NAG * L + j) % NPHYS


def _build_bass():
    import concourse.bass as bass
    import concourse.bacc as bacc
    import concourse.mybir as mybir
    import concourse.tile as tile

    dt = mybir.dt
    bf16 = dt.bfloat16
    f32 = dt.float32
    ts = bass.ts

    nc = bacc.Bacc(
        "TRN2",
        target_bir_lowering=False,
        debug=False,
        enable_asserts=False,
        num_devices=NCORES,
    )

    f8 = dt.float8e4

    xTf_d = nc.dram_tensor(
        "xTf", [NXBLK, P, XBLK, D0_IN // P, P], bf16, kind="ExternalInput"
    )
    # pre-tiled adjacency (e4m3), 512KB blocks of 2 groups x [P, KSUB, nw]
    adjT01_d = nc.dram_tensor(
        "adjT01", [2, NBLK, P, 2, KSUB, 512], f8, kind="ExternalInput"
    )
    adjT2_d = nc.dram_tensor(
        "adjT2", [NBLK, P, 2, KSUB, 256], f8, kind="ExternalInput"
    )
    W_d = [
        nc.dram_tensor("W0", [P, D0_IN // P, D0_OUT], bf16, kind="ExternalInput"),
        nc.dram_tensor("W1", [P, D1_IN // P, D1_OUT], bf16, kind="ExternalInput"),
        nc.dram_tensor("W2", [P, D2_IN // P, D2_OUT], bf16, kind="ExternalInput"),
    ]
    b_d = [
        nc.dram_tensor("b0", [P, D0_OUT // P], f32, kind="ExternalInput"),
        nc.dram_tensor("b1", [P, D1_OUT // P], f32, kind="ExternalInput"),
        nc.dram_tensor("b2", [P, D2_OUT // P], f32, kind="ExternalInput"),
    ]
    # [p, t, r]: feature f = t*P + p of output row r (decoded host-side)
    outT_d = nc.dram_tensor("outT", [P, D2_OUT // P, R], f32, kind="ExternalOutput")

    DIMS = [(D0_IN, D0_OUT), (D1_IN, D1_OUT), (D2_IN, D2_OUT)]

    with tile.TileContext(nc) as tc:
        ctx_pools = (
            tc.tile_pool(name="persist", bufs=1),
            tc.tile_pool(name="work", bufs=3),
            tc.tile_pool(name="psum", bufs=1, space="PSUM"),
            tc.tile_pool(name="dram", bufs=1, space="DRAM"),
        )
        with ctx_pools[0] as persist, ctx_pools[1] as work, \
             ctx_pools[2] as psum_pool, ctx_pools[3] as dram_pool:

            # ---- persistent tiles ----
            w_sb = [
                persist.tile([P, d_in // P, d_out], bf16, name=f"w{L}", tag=f"w{L}")
                for L, (d_in, d_out) in enumerate(DIMS)
            ]
            b_sb = [
                persist.tile([P, d_out // P], f32, name=f"b{L}", tag=f"b{L}")
                for L, (_, d_out) in enumerate(DIMS)
            ]
            # activations X^T (feature-major); xt1/xt2 lifetimes are
            # disjoint so they share one buffer
            xt = persist.tile([P, D1_IN // P, R], bf16, name="xt", tag="xt")
            xts = [None, xt, xt]

            # gathered S as (hi, lo) e4m3 pairs: 6 physical buffers rotating
            # over 5 chunks/layer
            s_ch = [
                persist.tile(
                    [P, AGT, 2, 512], f8, name=f"s_ch{b}", tag=f"s_ch{b}"
                )
                for b in range(NPHYS)
            ]

            s_bounce = [
                dram_pool.tile(
                    [R, 2, d_out], f8, name=f"s_bounce{L}", tag=f"sb{L}"
                )
                for L, (_, d_out) in enumerate(DIMS)
            ]
            s_all = [
                [
                    dram_pool.tile(
                        [NCORES * AGR, 2, d_out],
                        f8,
                        name=f"s_all{L}_{j}",
                        tag=f"sa{L}_{j}",
                        addr_space="Shared",
                    )
                    for j in range(NAG)
                ]
                for L, (_, d_out) in enumerate(DIMS)
            ]

            # ---- startup: first x blocks interleaved with w0, rest after ----
            xtiles = {}

            def load_xblk(blk):
                t = work.tile(
                    [P, XBLK, D0_IN // P, P], bf16, name=f"xtile_{blk}",
                    tag="xtile", bufs=3,
                )
                nc.scalar.dma_start(t[:], xTf_d[blk])
                xtiles[blk] = t

            load_xblk(0)
            nc.sync.dma_start(w_sb[0][:], W_d[0][:])
            load_xblk(1)
            nc.sync.dma_start(w_sb[1][:], W_d[1][:])
            nc.sync.dma_start(w_sb[2][:], W_d[2][:])
            for L in range(3):
                nc.sync.dma_start(b_sb[L][:], b_d[L][:])

            def dense_m(L, m):
                """dense S_k m-tile: psum = xt.T @ W, cast to bf16, to bounce."""
                d_in, d_out = DIMS[L]
                n_ct = d_in // P
                dps = psum_pool.tile(
                    [P, d_out], f32, name=f"dps_{L}_{m}", tag="dense_ps", bufs=3
                )
                for c in range(n_ct):
                    nc.tensor.matmul(
                        dps[:],
                        lhsT=xts[L][:, c, ts(m, P)],
                        rhs=w_sb[L][:, c, :],
                        start=(c == 0),
                        stop=(c == n_ct - 1),
                    )
                # dual-rail e4m3 split: hi = e4m3(S), lo = e4m3(S - hi)
                s_sb = work.tile(
                    [P, 2, d_out], f8, name=f"ssb_{L}_{m}", tag="s_sb", bufs=4
                )
                nc.vector.tensor_copy(s_sb[:, 0, :], dps[:])
                nc.vector.tensor_tensor(
                    s_sb[:, 1, :], dps[:], s_sb[:, 0, :],
                    mybir.AluOpType.subtract,
                )
                nc.scalar.dma_start(s_bounce[L][ts(m, P), :, :], s_sb[:])

            def ag_issue(L, j):
                """all-gather chunk j of layer L's S (writes s_all only)."""
                nc.gpsimd.collective_compute(
                    "AllGather",
                    mybir.AluOpType.bypass,
                    replica_groups=[list(range(NCORES))],
                    ins=[s_bounce[L][ts(j, AGR), :].opt()],
                    outs=[s_all[L][j].opt()],
                )

            def s_load(L, j):
                """load gathered chunk j into its physical buffer. Emitted
                after the program-order readers of that buffer (layer L-1)."""
                d_out = DIMS[L][1]
                buf = s_ch[_phys(L, j)]
                if d_out == 512:
                    src = s_all[L][j].rearrange("(t p) two d -> p t two d", p=P)
                    nc.scalar.dma_start(buf[:], src)
                else:
                    # d_out=256: pack the (hi, lo) pair contiguously into the
                    # first 512 bytes of each tile line (3-dim-balanceable)
                    src = s_all[L][j].rearrange("(t p) two d -> p t (two d)", p=P)
                    nc.scalar.dma_start(buf[:, :, 0, :], src)

            def spmm_nc(L, nci, sink):
                """SpMM n-chunk nci of layer L + epilogue via sink()."""
                d_out = DIMS[L][1]
                n_po = d_out // P
                n0, nw = N_CHUNKS[nci]
                sp_ps = [
                    psum_pool.tile(
                        [P, nw], f32, name=f"sp_{L}_{nci}_{p}", tag=f"sp{p}"
                    )
                    for p in range(n_po)
                ]
                for b in range(NBLK):
                    if nci < 2:
                        at = work.tile(
                            [P, 2, KSUB, 512], dt.float8e4,
                            name=f"at_{L}_{nci}_{b}", tag="at", bufs=3,
                        )
                        nc.sync.dma_start(at[:], adjT01_d[nci, b])
                    else:
                        at = work.tile(
                            [P, 2, KSUB, 256], dt.float8e4,
                            name=f"at2_{L}_{b}", tag="at2", bufs=3,
                        )
                        nc.sync.dma_start(at[:], adjT2_d[b])
                    for g2 in range(2):
                        gg = 2 * b + g2
                        jc = gg // 4  # all-gather chunk of this group
                        tbase = (gg % 4) * KSUB
                        sch = s_ch[_phys(L, jc)]
                        for s in range(KSUB):
                            adj = at[:, g2, s, :]
                            rhs = adj[:, None, :].broadcast_to((P, 2, nw))
                            if d_out == 512:
                                pair = sch[:, tbase + s, :, :]
                            else:
                                pair = sch[:, tbase + s, 0, :].rearrange(
                                    "p (two d) -> p two d", two=2
                                )
                            for p in range(n_po):
                                nc.tensor.matmul(
                                    sp_ps[p][:],
                                    lhsT=pair[:, :, ts(p, P)],
                                    rhs=rhs,
                                    start=(b == 0 and g2 == 0 and s == 0),
                                    stop=(
                                        b == NBLK - 1 and g2 == 1 and s == KSUB - 1
                                    ),
                                    perf_mode=mybir.MatmulPerfMode.DoubleRow,
                                )
                for p in range(n_po):
                    sink(p, sp_ps[p], n0, nw, n_po)

            def sink_mid(L):
                n_po_out = DIMS[L][1] // P

                def sink(p, ps, n0, nw, n_po):
                    # relu(x+b) on Vector (not ScalarE): the scalar queue
                    # head-blocks on s_load collective waits and would convoy
                    # an ACTIVATE sitting behind them
                    nc.vector.tensor_scalar(
                        xts[L + 1][:, p, n0 : n0 + nw],
                        ps[:],
                        b_sb[L][:, p : p + 1],
                        0.0,
                        op0=mybir.AluOpType.add,
                        op1=mybir.AluOpType.max,
                    )
                    nc.vector.tensor_scalar_add(
                        xts[L + 1][:, n_po_out + p, n0 : n0 + nw],
                        ps[:],
                        b_sb[L][:, p : p + 1],
                    )

                return sink

            def make_sink_out():
                state = {}

                def sink(p, ps, n0, nw, n_po):
                    if p == 0:
                        state["ot"] = work.tile(
                            [P, n_po, nw], f32, name=f"ot_{n0}", tag="ot", bufs=2
                        )
                    ot = state["ot"]
                    nc.vector.tensor_scalar_add(
                        ot[:, p, :], ps[:], b_sb[2][:, p : p + 1]
                    )
                    if p == n_po - 1:
                        nc.scalar.dma_start(outT_d[:, :, n0 : n0 + nw], ot[:])

                return sink

            # ================= pipeline =================
            # layer 0: every core computes the FULL S0 = x @ W0 locally
            # (redundant across cores) straight into s_ch -- no collective,
            # so the first all-gather is layer 1's, which has pipeline slack.
            for blk in range(NXBLK):
                if blk + 2 < NXBLK:
                    load_xblk(blk + 2)
                xtile = xtiles.pop(blk)
                for mi in range(XBLK):
                    mt = XBLK * blk + mi
                    dps = psum_pool.tile(
                        [P, D0_OUT], f32, name=f"dps0_{mt}", tag="dense_ps", bufs=3
                    )
                    for c in range(D0_IN // P):
                        nc.tensor.matmul(
                            dps[:],
                            lhsT=xtile[:, mi, c, :],
                            rhs=w_sb[0][:, c, :],
                            start=(c == 0),
                            stop=(c == D0_IN // P - 1),
                        )
                    sc = s_ch[_phys(0, mt // AGT)]
                    nc.vector.tensor_copy(sc[:, mt % AGT, 0, :], dps[:])
                    nc.vector.tensor_tensor(
                        sc[:, mt % AGT, 1, :], dps[:], sc[:, mt % AGT, 0, :],
                        mybir.AluOpType.subtract,
                    )
            # layer L spmm interleaved with layer L+1 dense + gather issue;
            # the s_ch loads must come after L's last spmm reads (program
            # order = Tile trace order), so they sit before L+1's spmm.
            for L in (0, 1):
                for nci in range(3):
                    spmm_nc(L, nci, sink_mid(L))
                    for m in M_OF_NC[nci]:
                        dense_m(L + 1, m)
                    for j in AG_OF_NC[nci]:
                        ag_issue(L + 1, j)
                for j in range(NAG):
                    s_load(L + 1, j)
            for nci in range(3):
                spmm_nc(2, nci, make_sink_out())

    nc.compile()
    return nc


def _get_nc():
    if "nc" not in _CACHE:
        _CACHE["nc"] = _build_bass()
    return _CACHE["nc"]


def _preprocess(x, edge_row, edge_col, edge_val, W0, W1, W2, b0, b1, b2):
    x = np.asarray(x, np.float32)
    edge_row = np.asarray(edge_row, np.int64)
    edge_col = np.asarray(edge_col, np.int64)
    edge_val = np.asarray(edge_val, np.float32)

    # contraction permutation for the chunked all-gather:
    # new index j*2048 + k*256 + r  <->  old (global node) k*1280 + j*256 + r
    jj, kk, rr = np.meshgrid(
        np.arange(NAG), np.arange(NCORES), np.arange(AGR), indexing="ij"
    )
    new_of_old = np.empty(NPAD, np.int64)
    new_of_old[(kk * R + jj * AGR + rr).ravel()] = (
        jj * (NCORES * AGR) + kk * AGR + rr
    ).ravel()

    # dense per-core adjacency blocks, transposed + permuted:
    # adjT[k][new_of_old[c], r_local] = sum of vals of edges (k*R+r_local, c)
    adjT = np.zeros((NCORES, NPAD, R), np.float32)
    core = edge_row // R
    r_local = edge_row % R
    np.add.at(adjT, (core, new_of_old[edge_col], r_local), edge_val)
    # x4 pre-scale centers the e4m3 mantissa range; 1/4 is folded into W
    adjT = (adjT * 4.0).astype(float8_e4m3fn)
    # [cores, NBLK, 2, KSUB, P, R] -> [cores, NBLK, P, 2, KSUB, R]
    a6 = adjT.reshape(NCORES, NBLK, 2, KSUB, P, R).transpose(0, 1, 4, 2, 3, 5)
    adjT01 = np.ascontiguousarray(
        np.stack([a6[..., 0:512], a6[..., 512:1024]], axis=1)
    )  # [cores, 2, NBLK, P, 2, KSUB, 512]
    adjT2 = np.ascontiguousarray(a6[..., 1024:1280])  # [cores, NBLK, P, 2, KSUB, 256]

    x_pad = np.zeros((NPAD, x.shape[1]), np.float32)
    x_pad[:N] = x
    old_of_new = np.empty(NPAD, np.int64)
    old_of_new[new_of_old] = np.arange(NPAD)
    # [blk, mi, n(row-in-tile), c, pf] -> [blk, pf, mi, c, n]
    xp5 = x_pad[old_of_new].reshape(NXBLK, XBLK, P, D0_IN // P, P)
    xTf = np.ascontiguousarray(xp5.transpose(0, 4, 1, 3, 2)).astype(bfloat16)

    def wtile(W):
        # 1/4 compensates the x4 adjacency pre-scale (exact in bf16)
        W = np.asarray(W, np.float32) * 0.25
        d_in, d_out = W.shape
        return np.ascontiguousarray(
            W.reshape(d_in // P, P, d_out).transpose(1, 0, 2)
        ).astype(bfloat16)

    def btile(b):
        b = np.asarray(b, np.float32)
        return np.ascontiguousarray(b.reshape(-1, P).T)

    in_maps = []
    for k in range(NCORES):
        in_maps.append(
            {
                "xTf": xTf,
                "adjT01": adjT01[k],
                "adjT2": adjT2[k],
                "W0": wtile(W0),
                "W1": wtile(W1),
                "W2": wtile(W2),
                "b0": btile(b0),
                "b1": btile(b1),
                "b2": btile(b2),
            }
        )
    return in_maps


def kernel(x, edge_row, edge_col, edge_val, W0, W1, W2, b0, b1, b2):
    global LAST_RESULT
    from concourse.bass_utils import run_bass_kernel_spmd

    nc = _get_nc()
    in_maps = _preprocess(
        x, edge_row, edge_col, edge_val, W0, W1, W2, b0, b1, b2
    )
    res = run_bass_kernel_spmd(
        nc,
        in_maps,
        core_ids=list(range(NCORES)),
        trace=bool(int(os.environ.get("GCN_TRACE", "0"))),
    )
    LAST_RESULT = res

    # per-core outT is [P, 2, R]; feature f = t*P + p
    outT = np.concatenate(
        [
            np.asarray(res.results[k]["outT"]).transpose(1, 0, 2).reshape(D2_OUT, R)
            for k in range(NCORES)
        ],
        axis=1,
    )  # [256, 10240]
    return np.ascontiguousarray(outT.T[:N]).astype(np.float32)


# revision 29
# speedup vs baseline: 1.1755x; 1.0665x over previous
"""GCN (3-layer, skip-concat) on 8 Trainium2 NeuronCores.

Strategy (hardcoded for N=10000, E=320000, dims 512/512/256):
  - Row-partition nodes across 8 cores (1280 padded rows each, N padded
    to 10240).
  - The adjacency shard is densified on the host into A_k^T [10240, 1280]
    (bf16): SpMM becomes a dense matmul on TensorE.
  - Activations live feature-major (X^T) in SBUF. Per layer:
      S_k   = X^T.T @ W          (node-major [1280, d_out], PE)
      S     = AllGather(S_k)     (bf16, HBM collective, 5x2MB chunks --
                                  measured CC per-byte cost is best ~2MB)
      Y^T   = S_tiles^T @ A_k^T  (feature-major, PE; S tiles stationary)
      X' ^T = [relu(Y^T + b); (Y^T + b)]   (partition-axis concat, free)
  - Software pipelining via emission order: layer L+1's dense m-tiles and
    all-gather chunks are emitted between layer L's SpMM n-chunks.
  - Two HWDGE queues: adjacency streams on qSP (sync); x tiles, weights,
    bounce writes, gathered-S loads and output on qAct (scalar), so the
    collective feeds never queue behind adjacency prefetch.
  - 6 physical s_ch buffers rotate over 3x5 logical chunks so the first
    gathered-S load of each layer has no write-after-read dependency on
    the previous layer's SpMM (kills the transition head-blocking).
  - DMAs are batched (1MB adjacency blocks, 4-mtile x blocks, single-DMA
    weights/biases) because each dma_start costs ~0.6us of issue time on
    the issuing engine.
  - xt1/xt2 share one SBUF buffer (their lifetimes are disjoint).
  - SpMM runs in fp8 DoubleRow mode at 2x PE rate: the adjacency is e4m3
    (x4 pre-scale, 1/4 folded into W), and S rides dual-rail as
    (hi, lo) e4m3 pairs with hi+lo ~ bf16 precision, so only the
    adjacency quantization (~1% rms) hits accuracy. The rhs pair dim is
    a 0-stride broadcast of the same adjacency block, so fp8 also halves
    adjacency DMA traffic. Dense X@W matmuls stay bf16.
    (measured end-to-end rel err ~1.1e-2 vs the 2e-2 gate)
"""

import os
import numpy as np
from ml_dtypes import bfloat16, float8_e4m3fn

N = 10000
NPAD = 10240
NCORES = 8
R = NPAD // NCORES  # 1280 rows per core
P = 128
CT = NPAD // P  # 80 contraction tiles for the SpMM
KSUB = 4  # contraction subtiles per adjacency group
NGRP = CT // KSUB  # 20 groups
NBLK = NGRP // 2  # 10 two-group adjacency DMA blocks

NAG = 5  # all-gather chunks per layer
AGR = R // NAG  # 256 rows per rank per chunk
AGT = NPAD // NAG // P  # 16 gathered contraction tiles per chunk
NPHYS = 6  # physical s_ch buffers (one spare frees WAR at transitions)

D0_IN, D0_OUT = 512, 512
D1_IN, D1_OUT = 1024, 512
D2_IN, D2_OUT = 1024, 256

# n-chunks of the 1280-wide free dim (PSUM bank = 512 fp32)
N_CHUNKS = [(0, 512), (512, 512), (1024, 256)]
# dense m-tiles whose lhsT columns come from n-chunk i's epilogue
M_OF_NC = [(0, 1, 2, 3), (4, 5, 6, 7), (8, 9)]
# all-gather chunk j consumes dense m-tiles 2j, 2j+1
AG_OF_NC = [(0, 1), (2, 3), (4,)]

XBLK = 4  # m-tiles per x DMA block
NXBLK = CT // XBLK  # 20

_CACHE = {}
LAST_RESULT = None  # BassKernelResults of the most recent run (for test.py)


def _phys(L, j):
    return (NAG * L + j) % NPHYS


def _build_bass():
    import concourse.bass as bass
    import concourse.bacc as bacc
    import concourse.mybir as mybir
    import concourse.tile as tile

    dt = mybir.dt
    bf16 = dt.bfloat16
    f32 = dt.float32
    ts = bass.ts

    nc = bacc.Bacc(
        "TRN2",
        target_bir_lowering=False,
        debug=False,
        enable_asserts=False,
        num_devices=NCORES,
    )

    f8 = dt.float8e4

    xTf_d = nc.dram_tensor(
        "xTf", [NXBLK, P, XBLK, D0_IN // P, P], bf16, kind="ExternalInput"
    )
    # pre-tiled adjacency (e4m3), 512KB blocks of 2 groups x [P, KSUB, nw]
    adjT01_d = nc.dram_tensor(
        "adjT01", [2, NBLK, P, 2, KSUB, 512], f8, kind="ExternalInput"
    )
    adjT2_d = nc.dram_tensor(
        "adjT2", [NBLK, P, 2, KSUB, 256], f8, kind="ExternalInput"
    )
    W_d = [
        nc.dram_tensor("W0", [P, D0_IN // P, D0_OUT], bf16, kind="ExternalInput"),
        nc.dram_tensor("W1", [P, D1_IN // P, D1_OUT], bf16, kind="ExternalInput"),
        nc.dram_tensor("W2", [P, D2_IN // P, D2_OUT], bf16, kind="ExternalInput"),
    ]
    b_d = [
        nc.dram_tensor("b0", [P, D0_OUT // P], f32, kind="ExternalInput"),
        nc.dram_tensor("b1", [P, D1_OUT // P], f32, kind="ExternalInput"),
        nc.dram_tensor("b2", [P, D2_OUT // P], f32, kind="ExternalInput"),
    ]
    # [p, t, r]: feature f = t*P + p of output row r (decoded host-side)
    outT_d = nc.dram_tensor("outT", [P, D2_OUT // P, R], f32, kind="ExternalOutput")

    DIMS = [(D0_IN, D0_OUT), (D1_IN, D1_OUT), (D2_IN, D2_OUT)]

    with tile.TileContext(nc) as tc:
        ctx_pools = (
            tc.tile_pool(name="persist", bufs=1),
            tc.tile_pool(name="work", bufs=3),
            tc.tile_pool(name="psum", bufs=1, space="PSUM"),
            tc.tile_pool(name="dram", bufs=1, space="DRAM"),
        )
        with ctx_pools[0] as persist, ctx_pools[1] as work, \
             ctx_pools[2] as psum_pool, ctx_pools[3] as dram_pool:

            # ---- persistent tiles ----
            w_sb = [
                persist.tile([P, d_in // P, d_out], bf16, name=f"w{L}", tag=f"w{L}")
                for L, (d_in, d_out) in enumerate(DIMS)
            ]
            b_sb = [
                persist.tile([P, d_out // P], f32, name=f"b{L}", tag=f"b{L}")
                for L, (_, d_out) in enumerate(DIMS)
            ]
            # activations X^T (feature-major); xt1/xt2 lifetimes are
            # disjoint so they share one buffer
            xt = persist.tile([P, D1_IN // P, R], bf16, name="xt", tag="xt")
            xts = [None, xt, xt]

            # gathered S as (hi, lo) e4m3 pairs: 6 physical buffers rotating
            # over 5 chunks/layer
            s_ch = [
                persist.tile(
                    [P, AGT, 2, 512], f8, name=f"s_ch{b}", tag=f"s_ch{b}"
                )
                for b in range(NPHYS)
            ]

            # bounce/gather layout is swizzled for coarse s_load DMA lines:
            # chunk j holds [P, 2(half), 2(rail), d] with row (p, half) <->
            # local row half*128+p, so the gathered s_all[j] = [k, p, 4*d]
            # loads into s_ch with one 2KB-contiguous line per (p, k).
            s_bounce = [
                dram_pool.tile(
                    [NAG, P, 4 * d_out], f8, name=f"s_bounce{L}", tag=f"sb{L}"
                )
                for L, (_, d_out) in enumerate(DIMS)
            ]
            s_all = [
                [
                    dram_pool.tile(
                        [NCORES, P, 4 * d_out],
                        f8,
                        name=f"s_all{L}_{j}",
                        tag=f"sa{L}_{j}",
                        addr_space="Shared",
                    )
                    for j in range(NAG)
                ]
                for L, (_, d_out) in enumerate(DIMS)
            ]

            # ---- startup: first x blocks interleaved with w0, rest after ----
            xtiles = {}

            def load_xblk(blk):
                t = work.tile(
                    [P, XBLK, D0_IN // P, P], bf16, name=f"xtile_{blk}",
                    tag="xtile", bufs=3,
                )
                nc.scalar.dma_start(t[:], xTf_d[blk])
                xtiles[blk] = t

            load_xblk(0)
            nc.sync.dma_start(w_sb[0][:], W_d[0][:])
            load_xblk(1)
            nc.sync.dma_start(w_sb[1][:], W_d[1][:])
            nc.sync.dma_start(w_sb[2][:], W_d[2][:])
            for L in range(3):
                nc.sync.dma_start(b_sb[L][:], b_d[L][:])

            def dense_m(L, m):
                """dense S_k m-tile: psum = xt.T @ W, cast to bf16, to bounce."""
                d_in, d_out = DIMS[L]
                n_ct = d_in // P
                dps = psum_pool.tile(
                    [P, d_out], f32, name=f"dps_{L}_{m}", tag="dense_ps", bufs=2
                )
                for c in range(n_ct):
                    nc.tensor.matmul(
                        dps[:],
                        lhsT=xts[L][:, c, ts(m, P)],
                        rhs=w_sb[L][:, c, :],
                        start=(c == 0),
                        stop=(c == n_ct - 1),
                    )
                # dual-rail e4m3 split: hi = e4m3(S), lo = e4m3(S - hi)
                s_sb = work.tile(
                    [P, 2, d_out], f8, name=f"ssb_{L}_{m}", tag="s_sb", bufs=4
                )
                nc.vector.tensor_copy(s_sb[:, 0, :], dps[:])
                nc.vector.tensor_tensor(
                    s_sb[:, 1, :], dps[:], s_sb[:, 0, :],
                    mybir.AluOpType.subtract,
                )
                # m-tile m = chunk m//2, half m%2 of the swizzled bounce
                dst = s_bounce[L][m // 2, :, (m % 2) * 2 * d_out :
                                  (m % 2 + 1) * 2 * d_out]
                nc.scalar.dma_start(dst, s_sb[:])

            def ag_issue(L, j):
                """all-gather chunk j of layer L's S (writes s_all only)."""
                nc.gpsimd.collective_compute(
                    "AllGather",
                    mybir.AluOpType.bypass,
                    replica_groups=[list(range(NCORES))],
                    ins=[s_bounce[L][j].opt()],
                    outs=[s_all[L][j].opt()],
                )

            def s_load(L, j):
                """load gathered chunk j into its physical buffer. Emitted
                after the program-order readers of that buffer (layer L-1)."""
                d_out = DIMS[L][1]
                buf = s_ch[_phys(L, j)]
                src = s_all[L][j].rearrange("k p x -> p k x")
                if d_out == 512:
                    # tile t = 2k+h; (h, rail, d) = 2KB contiguous per (p, k)
                    nc.scalar.dma_start(buf[:], src)
                else:
                    # d_out=256: tiles (2k, 2k+1) pack into line k (1KB lines)
                    nc.scalar.dma_start(buf[:, :NCORES, :, :], src)

            def spmm_nc(L, nci, sink):
                """SpMM n-chunk nci of layer L + epilogue via sink()."""
                d_out = DIMS[L][1]
                n_po = d_out // P
                n0, nw = N_CHUNKS[nci]
                sp_ps = [
                    psum_pool.tile(
                        [P, nw], f32, name=f"sp_{L}_{nci}_{p}", tag=f"sp{p}"
                    )
                    for p in range(n_po)
                ]
                for b in range(NBLK):
                    if nci < 2:
                        at = work.tile(
                            [P, 2, KSUB, 512], dt.float8e4,
                            name=f"at_{L}_{nci}_{b}", tag="at", bufs=3,
                        )
                        nc.sync.dma_start(at[:], adjT01_d[nci, b])
                    else:
                        at = work.tile(
                            [P, 2, KSUB, 256], dt.float8e4,
                            name=f"at2_{L}_{b}", tag="at2", bufs=3,
                        )
                        nc.sync.dma_start(at[:], adjT2_d[b])
                    for g2 in range(2):
                        gg = 2 * b + g2
                        jc = gg // 4  # all-gather chunk of this group
                        tbase = (gg % 4) * KSUB
                        sch = s_ch[_phys(L, jc)]
                        for s in range(KSUB):
                            adj = at[:, g2, s, :]
                            rhs = adj[:, None, :].broadcast_to((P, 2, nw))
                            t = tbase + s
                            if L == 0 or d_out == 512:
                                pair = sch[:, t, :, :]
                            else:
                                # L2 line-packed layout: tile t lives at
                                # (line t//2, half t%2) as (rail, 256)
                                pair = sch[:, t // 2, t % 2, :].rearrange(
                                    "p (two d) -> p two d", two=2
                                )
                            for p in range(n_po):
                                nc.tensor.matmul(
                                    sp_ps[p][:],
                                    lhsT=pair[:, :, ts(p, P)],
                                    rhs=rhs,
                                    start=(b == 0 and g2 == 0 and s == 0),
                                    stop=(
                                        b == NBLK - 1 and g2 == 1 and s == KSUB - 1
                                    ),
                                    perf_mode=mybir.MatmulPerfMode.DoubleRow,
                                )
                for p in range(n_po):
                    sink(p, sp_ps[p], n0, nw, n_po)

            def sink_mid(L):
                n_po_out = DIMS[L][1] // P

                def sink(p, ps, n0, nw, n_po):
                    # relu(x+b) on Vector (not ScalarE): the scalar queue
                    # head-blocks on s_load collective waits and would convoy
                    # an ACTIVATE sitting behind them
                    nc.vector.tensor_scalar(
                        xts[L + 1][:, p, n0 : n0 + nw],
                        ps[:],
                        b_sb[L][:, p : p + 1],
                        0.0,
                        op0=mybir.AluOpType.add,
                        op1=mybir.AluOpType.max,
                    )
                    nc.vector.tensor_scalar_add(
                        xts[L + 1][:, n_po_out + p, n0 : n0 + nw],
                        ps[:],
                        b_sb[L][:, p : p + 1],
                    )

                return sink

            def spmm_l2_fused():
                """Layer-2 SpMM: one contraction pass over all three
                1280-row n-chunks (6 PSUM banks, no interleaved dense),
                sharing each stationary across 3 matmuls. Gathered chunks
                are then consumed over the whole ~80us phase, relaxing the
                all-gather train deadlines by 3x."""
                n_po = 2
                ps = {}
                for nci, (n0, nw) in enumerate(N_CHUNKS):
                    for p in range(n_po):
                        tag = f"sp{nci * 2 + p}" if nci < 2 else f"spc{p}"
                        ps[(nci, p)] = psum_pool.tile(
                            [P, nw], f32, name=f"spf_{nci}_{p}", tag=tag
                        )
                for b in range(NBLK):
                    atf = work.tile(
                        [P, 2, KSUB, 1280], dt.float8e4,
                        name=f"atf_{b}", tag="atf", bufs=2,
                    )
                    nc.sync.dma_start(atf[:, :, :, 0:512], adjT01_d[0, b])
                    nc.sync.dma_start(atf[:, :, :, 512:1024], adjT01_d[1, b])
                    nc.sync.dma_start(atf[:, :, :, 1024:1280], adjT2_d[b])
                    for g2 in range(2):
                        gg = 2 * b + g2
                        jc = gg // 4
                        tbase = (gg % 4) * KSUB
                        sch = s_ch[_phys(2, jc)]
                        for s in range(KSUB):
                            t = tbase + s
                            pair = sch[:, t // 2, t % 2, :].rearrange(
                                "p (two d) -> p two d", two=2
                            )
                            for p in range(n_po):
                                lhsT = pair[:, :, ts(p, P)]
                                for nci, (n0, nw) in enumerate(N_CHUNKS):
                                    adj = atf[:, g2, s, n0 : n0 + nw]
                                    rhs = adj[:, None, :].broadcast_to(
                                        (P, 2, nw)
                                    )
                                    nc.tensor.matmul(
                                        ps[(nci, p)][:],
                                        lhsT=lhsT,
                                        rhs=rhs,
                                        start=(b == 0 and g2 == 0 and s == 0),
                                        stop=(
                                            b == NBLK - 1
                                            and g2 == 1
                                            and s == KSUB - 1
                                        ),
                                        perf_mode=mybir.MatmulPerfMode.DoubleRow,
                                    )
                for nci, (n0, nw) in enumerate(N_CHUNKS):
                    ot = work.tile(
                        [P, n_po, nw], f32, name=f"otf_{nci}", tag="ot", bufs=2
                    )
                    for p in range(n_po):
                        nc.vector.tensor_scalar_add(
                            ot[:, p, :], ps[(nci, p)][:], b_sb[2][:, p : p + 1]
                        )
                    nc.scalar.dma_start(outT_d[:, :, n0 : n0 + nw], ot[:])

            # ================= pipeline =================
            # layer 0: every core computes the FULL S0 = x @ W0 locally
            # (redundant across cores) straight into s_ch -- no collective,
            # so the first all-gather is layer 1's, which has pipeline slack.
            for blk in range(NXBLK):
                if blk + 2 < NXBLK:
                    load_xblk(blk + 2)
                xtile = xtiles.pop(blk)
                for mi in range(XBLK):
                    mt = XBLK * blk + mi
                    dps = psum_pool.tile(
                        [P, D0_OUT], f32, name=f"dps0_{mt}", tag="dense_ps", bufs=2
                    )
                    for c in range(D0_IN // P):
                        nc.tensor.matmul(
                            dps[:],
                            lhsT=xtile[:, mi, c, :],
                            rhs=w_sb[0][:, c, :],
                            start=(c == 0),
                            stop=(c == D0_IN // P - 1),
                        )
                    sc = s_ch[_phys(0, mt // AGT)]
                    nc.vector.tensor_copy(sc[:, mt % AGT, 0, :], dps[:])
                    nc.vector.tensor_tensor(
                        sc[:, mt % AGT, 1, :], dps[:], sc[:, mt % AGT, 0, :],
                        mybir.AluOpType.subtract,
                    )
            # layer L spmm interleaved with layer L+1 dense + gather issue;
            # the s_ch loads must come after L's last spmm reads (program
            # order = Tile trace order), so they sit before L+1's spmm.
            for L in (0, 1):
                for nci in range(3):
                    spmm_nc(L, nci, sink_mid(L))
                    for m in M_OF_NC[nci]:
                        dense_m(L + 1, m)
                    for j in AG_OF_NC[nci]:
                        ag_issue(L + 1, j)
                for j in range(NAG):
                    s_load(L + 1, j)
            spmm_l2_fused()

    nc.compile()
    return nc


def _get_nc():
    if "nc" not in _CACHE:
        _CACHE["nc"] = _build_bass()
    return _CACHE["nc"]


def _preprocess(x, edge_row, edge_col, edge_val, W0, W1, W2, b0, b1, b2):
    x = np.asarray(x, np.float32)
    edge_row = np.asarray(edge_row, np.int64)
    edge_col = np.asarray(edge_col, np.int64)
    edge_val = np.asarray(edge_val, np.float32)

    # contraction permutation for the chunked all-gather:
    # new index j*2048 + k*256 + r  <->  old (global node) k*1280 + j*256 + r
    jj, kk, rr = np.meshgrid(
        np.arange(NAG), np.arange(NCORES), np.arange(AGR), indexing="ij"
    )
    new_of_old = np.empty(NPAD, np.int64)
    new_of_old[(kk * R + jj * AGR + rr).ravel()] = (
        jj * (NCORES * AGR) + kk * AGR + rr
    ).ravel()

    # dense per-core adjacency blocks, transposed + permuted:
    # adjT[k][new_of_old[c], r_local] = sum of vals of edges (k*R+r_local, c)
    adjT = np.zeros((NCORES, NPAD, R), np.float32)
    core = edge_row // R
    r_local = edge_row % R
    np.add.at(adjT, (core, new_of_old[edge_col], r_local), edge_val)
    # x4 pre-scale centers the e4m3 mantissa range; 1/4 is folded into W
    adjT = (adjT * 4.0).astype(float8_e4m3fn)
    # [cores, NBLK, 2, KSUB, P, R] -> [cores, NBLK, P, 2, KSUB, R]
    a6 = adjT.reshape(NCORES, NBLK, 2, KSUB, P, R).transpose(0, 1, 4, 2, 3, 5)
    adjT01 = np.ascontiguousarray(
        np.stack([a6[..., 0:512], a6[..., 512:1024]], axis=1)
    )  # [cores, 2, NBLK, P, 2, KSUB, 512]
    adjT2 = np.ascontiguousarray(a6[..., 1024:1280])  # [cores, NBLK, P, 2, KSUB, 256]

    x_pad = np.zeros((NPAD, x.shape[1]), np.float32)
    x_pad[:N] = x
    old_of_new = np.empty(NPAD, np.int64)
    old_of_new[new_of_old] = np.arange(NPAD)
    # [blk, mi, n(row-in-tile), c, pf] -> [blk, pf, mi, c, n]
    xp5 = x_pad[old_of_new].reshape(NXBLK, XBLK, P, D0_IN // P, P)
    xTf = np.ascontiguousarray(xp5.transpose(0, 4, 1, 3, 2)).astype(bfloat16)

    def wtile(W):
        # 1/4 compensates the x4 adjacency pre-scale (exact in bf16)
        W = np.asarray(W, np.float32) * 0.25
        d_in, d_out = W.shape
        return np.ascontiguousarray(
            W.reshape(d_in // P, P, d_out).transpose(1, 0, 2)
        ).astype(bfloat16)

    def btile(b):
        b = np.asarray(b, np.float32)
        return np.ascontiguousarray(b.reshape(-1, P).T)

    in_maps = []
    for k in range(NCORES):
        in_maps.append(
            {
                "xTf": xTf,
                "adjT01": adjT01[k],
                "adjT2": adjT2[k],
                "W0": wtile(W0),
                "W1": wtile(W1),
                "W2": wtile(W2),
                "b0": btile(b0),
                "b1": btile(b1),
                "b2": btile(b2),
            }
        )
    return in_maps


def kernel(x, edge_row, edge_col, edge_val, W0, W1, W2, b0, b1, b2):
    global LAST_RESULT
    from concourse.bass_utils import run_bass_kernel_spmd

    nc = _get_nc()
    in_maps = _preprocess(
        x, edge_row, edge_col, edge_val, W0, W1, W2, b0, b1, b2
    )
    res = run_bass_kernel_spmd(
        nc,
        in_maps,
        core_ids=list(range(NCORES)),
        trace=bool(int(os.environ.get("GCN_TRACE", "0"))),
    )
    LAST_RESULT = res

    # per-core outT is [P, 2, R]; feature f = t*P + p
    outT = np.concatenate(
        [
            np.asarray(res.results[k]["outT"]).transpose(1, 0, 2).reshape(D2_OUT, R)
            for k in range(NCORES)
        ],
        axis=1,
    )  # [256, 10240]
    return np.ascontiguousarray(outT.T[:N]).astype(np.float32)
